# revision 47
# baseline (speedup 1.0000x reference)
"""AttnBlock (GroupNorm + single-head 1x1-conv attention + residual) on 8
Trainium2 NeuronCores, data-parallel over the batch dimension (one image per
core, weights replicated).

Per-core dataflow (x: [512 ch, 1024 px], all fp32):
  GN stats   : per-channel sum (DVE) + sum-of-squares (ACT Square accum)
               -> group sums via indicator matmul on PE
               -> finalize on 8 partitions -> broadcast back via padded
               indicator matmul -> h = x*a + b (fused affine, DVE)
  q/k        : [c,hw] layout, psum = wT.T @ h accumulated over 4 c-tiles,
               bias (+1/sqrt(c) folded into q) applied on PSUM->SBUF copy
  vT         : [hw,c] layout directly (lhsT = h tile), bias via broadcast add
  S^T = k^T q: [j,i] layout so softmax denominator is a ones-matmul and the
               second attention matmul needs no transposes; exp on ScalarE
               (scores are O(1) -- max-subtraction mathematically cancels and
               is numerically unnecessary here)
  U = v expS^T, y = wp.T @ U; out = x + y*recip(denom) + bproj, where the
               softmax normalization is deferred through the (column-scale
               commuting) projection matmul.
"""

from contextlib import ExitStack

import numpy as np

import concourse.bass as bass
import concourse.tile as tile
from concourse import mybir
from concourse.bass_utils import run_bass_kernel_spmd
from concourse.vector_clock import ScopedClock

B, C, HH, WW = 8, 512, 32, 32
HW = HH * WW          # 1024 pixels
P = 128               # SBUF partitions
CT = C // P           # 4 channel tiles
JT = HW // P          # 8 pixel tiles (keys)
NB = 512              # matmul moving free dim (one PSUM bank of fp32)
IB = HW // NB         # 2 query blocks
NG = 8                # groupnorm groups
GS = C // NG          # 64 channels per group
EPS = 1e-5
SCALE = float(1.0 / np.sqrt(np.float32(C)))

F32 = mybir.dt.float32
# float32r streams fp32 through the PE at full rate for free dims >= 256
# (plain float32 matmul costs 4 cycles/row), at ~2^-13 mantissa precision.
# Tiles feeding matmuls are allocated in this dtype so their producers
# (DVE/ACT writes, DMA loads) round once, for free.
MM_DT = mybir.dt.float32r


class _TC(tile.TileContext):
    """This container's walrus build rejects instructions carrying more than
    one sync-wait condition. After scheduling, hoist the extra waits of every
    multi-wait instruction into single-wait EventSemaphore instructions
    inserted just before it on the same engine (semantically identical)."""

    def _split_multiwait(self):
        nc = self.nc
        for bb in nc.main_func.blocks:
            insts = bb.instructions
            out = []
            changed = False
            for inst in insts:
                si = inst.sync_info
                if si is not None and si.on_wait and len(si.on_wait) > 1:
                    waits = list(si.on_wait)
                    si.on_wait = [waits[-1]]
                    for w in waits[:-1]:
                        wi = mybir.InstEventSemaphore(
                            name=nc.get_next_instruction_name()
                        )
                        wi.engine = inst.engine
                        wi.sync_info = mybir.SyncInfo(on_wait=[w], on_update=[])
                        out.append(wi)
                    changed = True
                out.append(inst)
            if changed:
                bb.instructions = out

    def _drain_and_barrier(self, tick_clock, wait_clock):
        nc = self.nc
        drain_inst = nc.sync.drain()
        wait_clock.add_sem_waits(
            drain_inst.ins, ScopedClock({None: tick_clock.global_clock})
        )
        self._split_multiwait()
        popped = nc._tile_sem_poison_stack.pop()
        assert popped is self._sem_poison


def _build():
    nc = bass.Bass()
    x = nc.dram_tensor("x", [C, HW], F32, kind="ExternalInput")
    wq = nc.dram_tensor("wqT", [C, C], MM_DT, kind="ExternalInput")
    wk = nc.dram_tensor("wkT", [C, C], MM_DT, kind="ExternalInput")
    wv = nc.dram_tensor("wvT", [C, C], MM_DT, kind="ExternalInput")
    wp = nc.dram_tensor("wpT", [C, C], MM_DT, kind="ExternalInput")
    bv = nc.dram_tensor("bv", [C], F32, kind="ExternalInput")
    smallcat = nc.dram_tensor(
        "smallcat", [P, 5 * CT + CT * NG + CT * P], F32, kind="ExternalInput"
    )
    out = nc.dram_tensor("out", [C, HW], F32, kind="ExternalOutput")

    with _TC(nc) as tc, ExitStack() as ctx:
        big = ctx.enter_context(tc.tile_pool(name="big", bufs=1))
        small = ctx.enter_context(tc.tile_pool(name="small", bufs=1))
        tmp = ctx.enter_context(tc.tile_pool(name="tmp", bufs=4))
        ps_pool = ctx.enter_context(tc.tile_pool(name="ps", bufs=7, space="PSUM"))
        ps_small = ctx.enter_context(tc.tile_pool(name="pss", bufs=1, space="PSUM"))
        outp = ctx.enter_context(tc.tile_pool(name="outp", bufs=4))

        # ---- loads ----
        # one DMA for all small constants; x per-tile so GN starts early;
        # weights spread across four engines' DMA queues to load in parallel
        sc_sb = small.tile([P, 5 * CT + CT * NG + CT * P], F32, tag="smallcat")
        xsb = big.tile([P, CT, HW], F32, tag="xsb")
        xr = x.rearrange("(t p) i -> p t i", p=P)
        # full-tile transfers keep 4KB descriptor rows (~2x the per-queue
        # bandwidth of split tiles); sync+scalar HWDGE each drain two
        # hardware queues concurrently
        nc.sync.dma_start(out=xsb[:, 0, :], in_=xr[:, 0, :])
        nc.sync.dma_start(out=xsb[:, 2, :], in_=xr[:, 2, :])
        nc.scalar.dma_start(out=xsb[:, 1, :], in_=xr[:, 1, :])
        nc.scalar.dma_start(out=xsb[:, 3, :], in_=xr[:, 3, :])
        nc.sync.dma_start(out=sc_sb[:], in_=smallcat[:])
        bq_sb = sc_sb[:, 0 * CT : 1 * CT]
        bk_sb = sc_sb[:, 1 * CT : 2 * CT]
        bp_sb = sc_sb[:, 2 * CT : 3 * CT]
        gs_sb = sc_sb[:, 3 * CT : 4 * CT]
        gb_sb = sc_sb[:, 4 * CT : 5 * CT]
        gmat_sb = sc_sb[:, 5 * CT : 5 * CT + CT * NG].rearrange(
            "p (t g) -> p t g", t=CT
        )
        hmat_sb = sc_sb[:, 5 * CT + CT * NG :].rearrange("p (t q) -> p t q", t=CT)
        wq_sb = big.tile([P, CT, C], MM_DT, tag="wq")
        nc.gpsimd.dma_start(out=wq_sb[:], in_=wq.rearrange("(t p) o -> p t o", p=P))
        wk_sb = big.tile([P, CT, C], MM_DT, tag="wk")
        nc.scalar.dma_start(out=wk_sb[:], in_=wk.rearrange("(t p) o -> p t o", p=P))
        wv_sb = big.tile([P, CT, C], MM_DT, tag="wv")
        nc.scalar.dma_start(out=wv_sb[:], in_=wv.rearrange("(t p) o -> p t o", p=P))
        wp_sb = big.tile([P, CT, C], MM_DT, tag="wp")
        nc.gpsimd.dma_start(out=wp_sb[:], in_=wp.rearrange("(t p) o -> p t o", p=P))
        bv_sb = small.tile([P, C], F32, tag="bv")
        nc.sync.dma_start(
            out=bv_sb[:],
            in_=bass.AP(tensor=bv[:].tensor, offset=0, ap=[[0, P], [1, C]]),
        )

        # memset doesn't encode for f32r; memset fp32 then round via copy
        ones_f32 = small.tile([P, 1], F32, tag="ones32")
        nc.vector.memset(ones_f32[:], 1.0)
        ones_sb = small.tile([P, 1], MM_DT, tag="ones")
        nc.vector.tensor_copy(out=ones_sb[:], in_=ones_f32[:])
        eps_sb = small.tile([P, 1], F32, tag="eps")
        nc.vector.memset(eps_sb[:], EPS)
        warm = tmp.tile([1, 1], F32, tag="actwarm")
        nc.scalar.activation(
            out=warm[:], in_=ones_f32[0:1, :],
            func=mybir.ActivationFunctionType.Square,
        )

        est = big.tile([P, JT, HW], MM_DT, tag="est")  # exp(S^T), [j, i]
        # ---- groupnorm statistics ----
        # raw per-channel sums: sum(x) on GpSimd, sum(x^2) via ACT Square
        # accumulator (est is dead until S^T and serves as the Square
        # scratch output); DVE stays free for the projection epilogues
        mom = small.tile([P, CT, 2], F32, tag="mom")  # (sum, sum(x^2))
        for t in range(CT):
            nc.vector.reduce_sum(
                out=mom[:, t, 0:1], in_=xsb[:, t, :], axis=mybir.AxisListType.X
            )
            nc.scalar.activation(
                out=est[:, t, :], in_=xsb[:, t, :],
                func=mybir.ActivationFunctionType.Square,
                accum_out=mom[:, t, 1:2],
            )

        # group sums across partitions: [8, 2] = gmat.T @ mom
        ps_g = ps_small.tile([NG, 2], F32, tag="pssmall")
        for t in range(CT):
            nc.tensor.matmul(
                ps_g[:], gmat_sb[:, t, :], mom[:, t, :],
                start=(t == 0), stop=(t == CT - 1),
            )

        # finalize on 8 partitions: mu = S1/64, var = S2/64 - mu^2,
        # rstd = 1/sqrt(var+eps); gsf rows 8..127 stay zero for the
        # K=128 broadcast matmul.
        sc = tmp.tile([P, 4], F32, tag="gnsc")
        nc.vector.tensor_scalar_mul(sc[0:NG, 0:1], ps_g[0:NG, 0:1], 1.0 / (GS * HW))
        nc.vector.tensor_scalar_mul(sc[0:NG, 1:2], ps_g[0:NG, 1:2], 1.0 / (GS * HW))
        nc.vector.tensor_tensor(
            out=sc[0:NG, 2:3], in0=sc[0:NG, 0:1], in1=sc[0:NG, 0:1],
            op=mybir.AluOpType.mult,
        )
        nc.vector.tensor_tensor(
            out=sc[0:NG, 3:4], in0=sc[0:NG, 1:2], in1=sc[0:NG, 2:3],
            op=mybir.AluOpType.subtract,
        )
        nc.scalar.activation(
            out=sc[0:NG, 2:3], in_=sc[0:NG, 3:4],
            func=mybir.ActivationFunctionType.Sqrt, bias=eps_sb[0:NG, 0:1],
        )
        warm2 = tmp.tile([1, 1], F32, tag="actwarm")
        nc.scalar.activation(
            out=warm2[:], in_=ones_f32[0:1, :],
            func=mybir.ActivationFunctionType.Exp,
        )
        gsf = small.tile([P, 2], F32, tag="gsf")
        nc.vector.memset(gsf[:], 0.0)
        nc.vector.tensor_copy(out=gsf[0:NG, 0:1], in_=sc[0:NG, 0:1])
        nc.vector.reciprocal(out=gsf[0:NG, 1:2], in_=sc[0:NG, 2:3])

        # broadcast (mu, rstd) back to channel partitions; fold affine:
        # a = rstd*gn_scale ; b = gn_bias - mu*a ; h = x*a + b
        hsb = big.tile([P, CT, HW], MM_DT, tag="hsb")
        ab = small.tile([P, CT, 2], F32, tag="ab")
        for t in range(CT):
            ps_b = ps_small.tile([P, 2], F32, tag="pssmall")
            nc.tensor.matmul(
                ps_b[:], hmat_sb[:, t, :], gsf[:], start=True, stop=True
            )
            nc.vector.tensor_tensor(
                out=ab[:, t, 0:1], in0=ps_b[:, 1:2], in1=gs_sb[:, t : t + 1],
                op=mybir.AluOpType.mult,
            )
            nc.vector.tensor_tensor(
                out=ab[:, t, 1:2], in0=ps_b[:, 0:1], in1=ab[:, t, 0:1],
                op=mybir.AluOpType.mult,
            )
            nc.vector.tensor_tensor(
                out=ab[:, t, 1:2], in0=gb_sb[:, t : t + 1], in1=ab[:, t, 1:2],
                op=mybir.AluOpType.subtract,
            )
            nc.vector.tensor_scalar(
                out=hsb[:, t, :], in0=xsb[:, t, :],
                scalar1=ab[:, t, 0:1], scalar2=ab[:, t, 1:2],
                op0=mybir.AluOpType.mult, op1=mybir.AluOpType.add,
            )
            # residual base: xsb <- x + bproj (consumed by the final add)
            nc.vector.tensor_scalar_add(
                xsb[:, t, :], xsb[:, t, :], bp_sb[:, t : t + 1]
            )

        # ---- q / k projections, [c, hw] layout ----
        q_sb = big.tile([P, CT, HW], MM_DT, tag="q")
        k_sb = big.tile([P, CT, HW], MM_DT, tag="k")
        for ot in range(CT):
            for ib in range(IB):
                isl = slice(ib * NB, (ib + 1) * NB)
                psq = ps_pool.tile([P, NB], F32, tag="mmps")
                for ct in range(CT):
                    nc.tensor.matmul(
                        psq[:],
                        wq_sb[:, ct, ot * P : (ot + 1) * P],
                        hsb[:, ct, isl],
                        start=(ct == 0), stop=(ct == CT - 1),
                    )
                # q = (wq.h + bq) * (1/sqrt(c)) -- attention scale folded in
                nc.vector.tensor_scalar(
                    out=q_sb[:, ot, isl], in0=psq[:],
                    scalar1=bq_sb[:, ot : ot + 1], scalar2=SCALE,
                    op0=mybir.AluOpType.add, op1=mybir.AluOpType.mult,
                )
                psk = ps_pool.tile([P, NB], F32, tag="mmps")
                for ct in range(CT):
                    nc.tensor.matmul(
                        psk[:],
                        wk_sb[:, ct, ot * P : (ot + 1) * P],
                        hsb[:, ct, isl],
                        start=(ct == 0), stop=(ct == CT - 1),
                    )
                nc.vector.tensor_scalar_add(
                    k_sb[:, ot, isl], psk[:], bk_sb[:, ot : ot + 1]
                )

        # ---- vT projection, [hw, c] layout ----
        vT_sb = big.tile([P, JT, C], MM_DT, tag="vT")
        for jt in range(JT):
            psv = ps_pool.tile([P, NB], F32, tag="mmps")
            for ct in range(CT):
                nc.tensor.matmul(
                    psv[:],
                    hsb[:, ct, jt * P : (jt + 1) * P],
                    wv_sb[:, ct, :],
                    start=(ct == 0), stop=(ct == CT - 1),
                )
            nc.vector.tensor_add(out=vT_sb[:, jt, :], in0=psv[:], in1=bv_sb[:])

        # ---- S^T = k^T q (scaled), exp; per-ib denominator right after ----
        recip_f32 = small.tile([1, HW], F32, tag="recip32")
        recip = small.tile([1, HW], MM_DT, tag="recip")
        ones_row_f32 = small.tile([1, P], F32, tag="onesr32")
        nc.vector.memset(ones_row_f32[:], 1.0)
        ones_row = small.tile([1, P], MM_DT, tag="onesr")
        nc.vector.tensor_copy(out=ones_row[:], in_=ones_row_f32[:])
        for ib in range(IB):
            isl = slice(ib * NB, (ib + 1) * NB)
            for jt in range(JT):
                pss = ps_pool.tile([P, NB], F32, tag="mmps")
                for ct in range(CT):
                    nc.tensor.matmul(
                        pss[:],
                        k_sb[:, ct, jt * P : (jt + 1) * P],
                        q_sb[:, ct, isl],
                        start=(ct == 0), stop=(ct == CT - 1),
                    )
                nc.scalar.activation(
                    out=est[:, jt, isl], in_=pss[:],
                    func=mybir.ActivationFunctionType.Exp,
                )
            # denom[i] = sum_j exp(S^T)[j, i] via ones-matmuls
            ps_d = ps_small.tile([1, NB], F32, tag="pssmall")
            for jt in range(JT):
                nc.tensor.matmul(
                    ps_d[:], ones_sb[:], est[:, jt, isl],
                    start=(jt == 0), stop=(jt == JT - 1),
                )
            nc.vector.reciprocal(out=recip_f32[:, isl], in_=ps_d[:])
            nc.vector.tensor_copy(out=recip[:, isl], in_=recip_f32[:, isl])

        # ---- U = v @ expS^T, normalization folded into the PSUM drain ----
        # the rep outer-product matmul is emitted after U's first group so
        # the in-order PE queue isn't head-of-line blocked on the reciprocal
        rep = big.tile([P, HW], F32, tag="rep")
        u_sb = big.tile([P, CT, HW], MM_DT, tag="u")
        for ib in range(IB):
            isl = slice(ib * NB, (ib + 1) * NB)
            for ct in range(CT):
                psu = ps_pool.tile([P, NB], F32, tag="mmps")
                for jt in range(JT):
                    nc.tensor.matmul(
                        psu[:],
                        vT_sb[:, jt, ct * P : (ct + 1) * P],
                        est[:, jt, isl],
                        start=(jt == 0), stop=(jt == JT - 1),
                    )
                if ct == 0:
                    ps_r = ps_small.tile([P, NB], F32, tag="pssmall")
                    nc.tensor.matmul(
                        ps_r[:], ones_row[:], recip[:, isl],
                        start=True, stop=True,
                    )
                    nc.vector.tensor_copy(out=rep[:, isl], in_=ps_r[:])
                nc.vector.tensor_tensor(
                    out=u_sb[:, ct, isl], in0=psu[:], in1=rep[:, isl],
                    op=mybir.AluOpType.mult,
                )

        # ---- proj + deferred softmax normalization + residual ----
        for ib in range(IB):
            for ot in range(CT):
                isl = slice(ib * NB, (ib + 1) * NB)
                psp = ps_pool.tile([P, NB], F32, tag="mmps")
                for ct in range(CT):
                    nc.tensor.matmul(
                        psp[:],
                        wp_sb[:, ct, ot * P : (ot + 1) * P],
                        u_sb[:, ct, isl],
                        start=(ct == 0), stop=(ct == CT - 1),
                    )
                ot_t = outp.tile([P, NB], F32, tag="out", name="ot_t")
                nc.vector.tensor_tensor(
                    out=ot_t[:], in0=psp[:], in1=xsb[:, ot, isl],
                    op=mybir.AluOpType.add,
                )
                [nc.sync, nc.scalar, nc.gpsimd, nc.sync][ot].dma_start(
                    out=out.rearrange("(t p) i -> p t i", p=P)[:, ot, isl],
                    in_=ot_t[:],
                )
    return nc


_NC = None


def _get_nc():
    global _NC
    if _NC is None:
        _NC = _build()
    return _NC


def _prep_inputs(x, gn_scale, gn_bias, wq, bq, wk, bk, wv, bv, wproj, bproj):
    f = np.float32
    x = np.ascontiguousarray(x, dtype=f).reshape(B, C, HW)

    def t2(w):  # [o, c] -> [c, o]
        return np.ascontiguousarray(np.asarray(w, dtype=f).T)

    def pt(v):  # [512] -> [128, 4] with v[t*128 + p] at [p, t]
        return np.ascontiguousarray(np.asarray(v, dtype=f).reshape(CT, P).T)

    pidx = np.arange(P)[:, None]
    tidx = np.arange(CT)[None, :]
    grp = 2 * tidx + pidx // GS  # [128, 4] group id per (p, t)
    gmat = np.zeros((P, CT, NG), f)
    hmat = np.zeros((P, CT, P), f)
    for t in range(CT):
        gmat[pidx[:, 0], t, grp[:, t]] = 1.0
        hmat[grp[:, t], t, pidx[:, 0]] = 1.0

    smallcat = np.concatenate(
        [
            pt(bq), pt(bk), pt(bproj), pt(gn_scale), pt(gn_bias),
            gmat.reshape(P, CT * NG), hmat.reshape(P, CT * P),
        ],
        axis=1,
    )
    shared = {
        "wqT": t2(wq), "wkT": t2(wk), "wvT": t2(wv), "wpT": t2(wproj),
        "bv": np.ascontiguousarray(np.asarray(bv, dtype=f)),
        "smallcat": np.ascontiguousarray(smallcat),
    }
    return [dict(shared, x=np.ascontiguousarray(x[b])) for b in range(B)]


def _run(inputs, **kw):
    nc = _get_nc()
    in_maps = _prep_inputs(**inputs)
    return run_bass_kernel_spmd(nc, in_maps, core_ids=list(range(B)), **kw)


def kernel(**inputs) -> np.ndarray:
    res = _run(inputs)
    out = np.stack([res.results[b]["out"] for b in range(B)])
    return out.reshape(B, C, HH, WW).astype(np.float32)


# revision 48
# speedup vs baseline: 1.0131x; 1.0131x over previous
"""AttnBlock (GroupNorm + single-head 1x1-conv attention + residual) on 8
Trainium2 NeuronCores, data-parallel over the batch dimension (one image per
core, weights replicated).

Per-core dataflow (x: [512 ch, 1024 px], all fp32):
  GN stats   : per-channel sum (DVE) + sum-of-squares (ACT Square accum)
               -> group sums via indicator matmul on PE
               -> finalize on 8 partitions -> broadcast back via padded
               indicator matmul -> h = x*a + b (fused affine, DVE)
  q/k        : [c,hw] layout, psum = wT.T @ h accumulated over 4 c-tiles,
               bias (+1/sqrt(c) folded into q) applied on PSUM->SBUF copy
  vT         : [hw,c] layout directly (lhsT = h tile), bias via broadcast add
  S^T = k^T q: [j,i] layout so softmax denominator is a ones-matmul and the
               second attention matmul needs no transposes; exp on ScalarE
               (scores are O(1) -- max-subtraction mathematically cancels and
               is numerically unnecessary here)
  U = v expS^T, y = wp.T @ U; out = x + y*recip(denom) + bproj, where the
               softmax normalization is deferred through the (column-scale
               commuting) projection matmul.
"""

from contextlib import ExitStack

import numpy as np

import concourse.bass as bass
import concourse.tile as tile
from concourse import mybir
from concourse.bass_utils import run_bass_kernel_spmd
from concourse.vector_clock import ScopedClock

B, C, HH, WW = 8, 512, 32, 32
HW = HH * WW          # 1024 pixels
P = 128               # SBUF partitions
CT = C // P           # 4 channel tiles
JT = HW // P          # 8 pixel tiles (keys)
NB = 512              # matmul moving free dim (one PSUM bank of fp32)
IB = HW // NB         # 2 query blocks
NG = 8                # groupnorm groups
GS = C // NG          # 64 channels per group
EPS = 1e-5
SCALE = float(1.0 / np.sqrt(np.float32(C)))

F32 = mybir.dt.float32
# float32r streams fp32 through the PE at full rate for free dims >= 256
# (plain float32 matmul costs 4 cycles/row), at ~2^-13 mantissa precision.
# Tiles feeding matmuls are allocated in this dtype so their producers
# (DVE/ACT writes, DMA loads) round once, for free.
MM_DT = mybir.dt.float32r


class _TC(tile.TileContext):
    """This container's walrus build rejects instructions carrying more than
    one sync-wait condition. After scheduling, hoist the extra waits of every
    multi-wait instruction into single-wait EventSemaphore instructions
    inserted just before it on the same engine (semantically identical)."""

    def _split_multiwait(self):
        nc = self.nc
        for bb in nc.main_func.blocks:
            insts = bb.instructions
            out = []
            changed = False
            for inst in insts:
                si = inst.sync_info
                if si is not None and si.on_wait and len(si.on_wait) > 1:
                    waits = list(si.on_wait)
                    si.on_wait = [waits[-1]]
                    for w in waits[:-1]:
                        wi = mybir.InstEventSemaphore(
                            name=nc.get_next_instruction_name()
                        )
                        wi.engine = inst.engine
                        wi.sync_info = mybir.SyncInfo(on_wait=[w], on_update=[])
                        out.append(wi)
                    changed = True
                out.append(inst)
            if changed:
                bb.instructions = out

    def _drain_and_barrier(self, tick_clock, wait_clock):
        nc = self.nc
        drain_inst = nc.sync.drain()
        wait_clock.add_sem_waits(
            drain_inst.ins, ScopedClock({None: tick_clock.global_clock})
        )
        self._split_multiwait()
        popped = nc._tile_sem_poison_stack.pop()
        assert popped is self._sem_poison


def _build():
    nc = bass.Bass()
    x = nc.dram_tensor("x", [C, HW], F32, kind="ExternalInput")
    wq = nc.dram_tensor("wqT", [C, C], MM_DT, kind="ExternalInput")
    wk = nc.dram_tensor("wkT", [C, C], MM_DT, kind="ExternalInput")
    wv = nc.dram_tensor("wvT", [C, C], MM_DT, kind="ExternalInput")
    wp = nc.dram_tensor("wpT", [C, C], MM_DT, kind="ExternalInput")
    bv = nc.dram_tensor("bv", [C], F32, kind="ExternalInput")
    smallcat = nc.dram_tensor(
        "smallcat", [P, 5 * CT + CT * NG + CT * P], F32, kind="ExternalInput"
    )
    out = nc.dram_tensor("out", [C, HW], F32, kind="ExternalOutput")

    with _TC(nc) as tc, ExitStack() as ctx:
        big = ctx.enter_context(tc.tile_pool(name="big", bufs=1))
        small = ctx.enter_context(tc.tile_pool(name="small", bufs=1))
        tmp = ctx.enter_context(tc.tile_pool(name="tmp", bufs=4))
        ps_pool = ctx.enter_context(tc.tile_pool(name="ps", bufs=7, space="PSUM"))
        ps_small = ctx.enter_context(tc.tile_pool(name="pss", bufs=1, space="PSUM"))
        outp = ctx.enter_context(tc.tile_pool(name="outp", bufs=4))

        # ---- loads ----
        # one DMA for all small constants; x per-tile so GN starts early;
        # weights spread across four engines' DMA queues to load in parallel
        sc_sb = small.tile([P, 5 * CT + CT * NG + CT * P], F32, tag="smallcat")
        xsb = big.tile([P, CT, HW], F32, tag="xsb")
        xr = x.rearrange("(t p) i -> p t i", p=P)
        # full-tile transfers keep 4KB descriptor rows (~2x the per-queue
        # bandwidth of split tiles); sync+scalar HWDGE each drain two
        # hardware queues concurrently
        nc.sync.dma_start(out=xsb[:, 0, :], in_=xr[:, 0, :])
        nc.sync.dma_start(out=xsb[:, 2, :], in_=xr[:, 2, :])
        nc.scalar.dma_start(out=xsb[:, 1, :], in_=xr[:, 1, :])
        nc.scalar.dma_start(out=xsb[:, 3, :], in_=xr[:, 3, :])
        nc.sync.dma_start(out=sc_sb[:], in_=smallcat[:])
        bq_sb = sc_sb[:, 0 * CT : 1 * CT]
        bk_sb = sc_sb[:, 1 * CT : 2 * CT]
        bp_sb = sc_sb[:, 2 * CT : 3 * CT]
        gs_sb = sc_sb[:, 3 * CT : 4 * CT]
        gb_sb = sc_sb[:, 4 * CT : 5 * CT]
        gmat_sb = sc_sb[:, 5 * CT : 5 * CT + CT * NG].rearrange(
            "p (t g) -> p t g", t=CT
        )
        hmat_sb = sc_sb[:, 5 * CT + CT * NG :].rearrange("p (t q) -> p t q", t=CT)
        wq_sb = big.tile([P, CT, C], MM_DT, tag="wq")
        nc.gpsimd.dma_start(out=wq_sb[:], in_=wq.rearrange("(t p) o -> p t o", p=P))
        wk_sb = big.tile([P, CT, C], MM_DT, tag="wk")
        nc.scalar.dma_start(out=wk_sb[:], in_=wk.rearrange("(t p) o -> p t o", p=P))
        wv_sb = big.tile([P, CT, C], MM_DT, tag="wv")
        nc.scalar.dma_start(out=wv_sb[:], in_=wv.rearrange("(t p) o -> p t o", p=P))
        wp_sb = big.tile([P, CT, C], MM_DT, tag="wp")
        nc.gpsimd.dma_start(out=wp_sb[:], in_=wp.rearrange("(t p) o -> p t o", p=P))
        bv_sb = small.tile([P, C], F32, tag="bv")
        nc.sync.dma_start(
            out=bv_sb[:],
            in_=bass.AP(tensor=bv[:].tensor, offset=0, ap=[[0, P], [1, C]]),
        )

        # memset doesn't encode for f32r; memset fp32 then round via copy
        ones_f32 = small.tile([P, 1], F32, tag="ones32")
        nc.vector.memset(ones_f32[:], 1.0)
        ones_sb = small.tile([P, 1], MM_DT, tag="ones")
        nc.vector.tensor_copy(out=ones_sb[:], in_=ones_f32[:])
        eps_sb = small.tile([P, 1], F32, tag="eps")
        nc.vector.memset(eps_sb[:], EPS)
        warm = tmp.tile([1, 1], F32, tag="actwarm")
        nc.scalar.activation(
            out=warm[:], in_=ones_f32[0:1, :],
            func=mybir.ActivationFunctionType.Square,
        )

        est = big.tile([P, JT, HW], MM_DT, tag="est")  # exp(S^T), [j, i]
        # ---- groupnorm statistics ----
        # raw per-channel sums: sum(x) on GpSimd, sum(x^2) via ACT Square
        # accumulator (est is dead until S^T and serves as the Square
        # scratch output); DVE stays free for the projection epilogues
        mom = small.tile([P, CT, 2], F32, tag="mom")  # (sum, sum(x^2))
        for t in range(CT):
            nc.vector.reduce_sum(
                out=mom[:, t, 0:1], in_=xsb[:, t, :], axis=mybir.AxisListType.X
            )
            nc.scalar.activation(
                out=est[:, t, :], in_=xsb[:, t, :],
                func=mybir.ActivationFunctionType.Square,
                accum_out=mom[:, t, 1:2],
            )

        # group sums across partitions: [8, 2] = gmat.T @ mom
        ps_g = ps_small.tile([NG, 2], F32, tag="pssmall")
        for t in range(CT):
            nc.tensor.matmul(
                ps_g[:], gmat_sb[:, t, :], mom[:, t, :],
                start=(t == 0), stop=(t == CT - 1),
            )

        # finalize on 8 partitions: mu = S1/64, var = S2/64 - mu^2,
        # rstd = 1/sqrt(var+eps); gsf rows 8..127 stay zero for the
        # K=128 broadcast matmul.
        sc = tmp.tile([P, 4], F32, tag="gnsc")
        nc.vector.tensor_scalar_mul(sc[0:NG, 0:1], ps_g[0:NG, 0:1], 1.0 / (GS * HW))
        nc.vector.tensor_scalar_mul(sc[0:NG, 1:2], ps_g[0:NG, 1:2], 1.0 / (GS * HW))
        nc.vector.tensor_tensor(
            out=sc[0:NG, 2:3], in0=sc[0:NG, 0:1], in1=sc[0:NG, 0:1],
            op=mybir.AluOpType.mult,
        )
        nc.vector.tensor_tensor(
            out=sc[0:NG, 3:4], in0=sc[0:NG, 1:2], in1=sc[0:NG, 2:3],
            op=mybir.AluOpType.subtract,
        )
        nc.scalar.activation(
            out=sc[0:NG, 2:3], in_=sc[0:NG, 3:4],
            func=mybir.ActivationFunctionType.Sqrt, bias=eps_sb[0:NG, 0:1],
        )
        gsf = small.tile([P, 2], F32, tag="gsf")
        nc.vector.memset(gsf[:], 0.0)
        nc.vector.tensor_copy(out=gsf[0:NG, 0:1], in_=sc[0:NG, 0:1])
        nc.vector.reciprocal(out=gsf[0:NG, 1:2], in_=sc[0:NG, 2:3])

        # broadcast (mu, rstd) back to channel partitions; fold affine:
        # a = rstd*gn_scale ; b = gn_bias - mu*a ; h = x*a + b
        hsb = big.tile([P, CT, HW], MM_DT, tag="hsb")
        ab = small.tile([P, CT, 2], F32, tag="ab")
        for t in range(CT):
            ps_b = ps_small.tile([P, 2], F32, tag="pssmall")
            nc.tensor.matmul(
                ps_b[:], hmat_sb[:, t, :], gsf[:], start=True, stop=True
            )
            nc.vector.tensor_tensor(
                out=ab[:, t, 0:1], in0=ps_b[:, 1:2], in1=gs_sb[:, t : t + 1],
                op=mybir.AluOpType.mult,
            )
            nc.vector.tensor_tensor(
                out=ab[:, t, 1:2], in0=ps_b[:, 0:1], in1=ab[:, t, 0:1],
                op=mybir.AluOpType.mult,
            )
            nc.vector.tensor_tensor(
                out=ab[:, t, 1:2], in0=gb_sb[:, t : t + 1], in1=ab[:, t, 1:2],
                op=mybir.AluOpType.subtract,
            )
            nc.vector.tensor_scalar(
                out=hsb[:, t, :], in0=xsb[:, t, :],
                scalar1=ab[:, t, 0:1], scalar2=ab[:, t, 1:2],
                op0=mybir.AluOpType.mult, op1=mybir.AluOpType.add,
            )
            # residual base: xsb <- x + bproj (consumed by the final add)
            nc.vector.tensor_scalar_add(
                xsb[:, t, :], xsb[:, t, :], bp_sb[:, t : t + 1]
            )

        # ---- q / k projections, [c, hw] layout ----
        q_sb = big.tile([P, CT, HW], MM_DT, tag="q")
        k_sb = big.tile([P, CT, HW], MM_DT, tag="k")
        for ot in range(CT):
            for ib in range(IB):
                isl = slice(ib * NB, (ib + 1) * NB)
                psq = ps_pool.tile([P, NB], F32, tag="mmps")
                for ct in range(CT):
                    nc.tensor.matmul(
                        psq[:],
                        wq_sb[:, ct, ot * P : (ot + 1) * P],
                        hsb[:, ct, isl],
                        start=(ct == 0), stop=(ct == CT - 1),
                    )
                # q = wq.h * (1/sqrt(c)) + bq/sqrt(c) -- attention scale
                # folded in; bq arrives pre-scaled from the host. ACT is idle
                # in this phase while DVE is nearly saturated.
                nc.scalar.activation(
                    out=q_sb[:, ot, isl], in_=psq[:],
                    func=mybir.ActivationFunctionType.Identity,
                    bias=bq_sb[:, ot : ot + 1], scale=SCALE,
                )
                psk = ps_pool.tile([P, NB], F32, tag="mmps")
                for ct in range(CT):
                    nc.tensor.matmul(
                        psk[:],
                        wk_sb[:, ct, ot * P : (ot + 1) * P],
                        hsb[:, ct, isl],
                        start=(ct == 0), stop=(ct == CT - 1),
                    )
                nc.scalar.activation(
                    out=k_sb[:, ot, isl], in_=psk[:],
                    func=mybir.ActivationFunctionType.Identity,
                    bias=bk_sb[:, ot : ot + 1],
                )

        # ---- vT projection, [hw, c] layout ----
        vT_sb = big.tile([P, JT, C], MM_DT, tag="vT")
        for jt in range(JT):
            psv = ps_pool.tile([P, NB], F32, tag="mmps")
            for ct in range(CT):
                nc.tensor.matmul(
                    psv[:],
                    hsb[:, ct, jt * P : (jt + 1) * P],
                    wv_sb[:, ct, :],
                    start=(ct == 0), stop=(ct == CT - 1),
                )
            nc.vector.tensor_add(out=vT_sb[:, jt, :], in0=psv[:], in1=bv_sb[:])

        # ---- S^T = k^T q (scaled), exp; per-ib denominator right after ----
        recip_f32 = small.tile([1, HW], F32, tag="recip32")
        recip = small.tile([1, HW], MM_DT, tag="recip")
        ones_row_f32 = small.tile([1, P], F32, tag="onesr32")
        nc.vector.memset(ones_row_f32[:], 1.0)
        ones_row = small.tile([1, P], MM_DT, tag="onesr")
        nc.vector.tensor_copy(out=ones_row[:], in_=ones_row_f32[:])
        for ib in range(IB):
            isl = slice(ib * NB, (ib + 1) * NB)
            for jt in range(JT):
                pss = ps_pool.tile([P, NB], F32, tag="mmps")
                for ct in range(CT):
                    nc.tensor.matmul(
                        pss[:],
                        k_sb[:, ct, jt * P : (jt + 1) * P],
                        q_sb[:, ct, isl],
                        start=(ct == 0), stop=(ct == CT - 1),
                    )
                nc.scalar.activation(
                    out=est[:, jt, isl], in_=pss[:],
                    func=mybir.ActivationFunctionType.Exp,
                )
            # denom[i] = sum_j exp(S^T)[j, i] via ones-matmuls
            ps_d = ps_small.tile([1, NB], F32, tag="pssmall")
            for jt in range(JT):
                nc.tensor.matmul(
                    ps_d[:], ones_sb[:], est[:, jt, isl],
                    start=(jt == 0), stop=(jt == JT - 1),
                )
            nc.vector.reciprocal(out=recip_f32[:, isl], in_=ps_d[:])
            nc.vector.tensor_copy(out=recip[:, isl], in_=recip_f32[:, isl])

        # ---- U = v @ expS^T, normalization folded into the PSUM drain ----
        # the rep outer-product matmul is emitted after U's first group so
        # the in-order PE queue isn't head-of-line blocked on the reciprocal
        rep = big.tile([P, HW], F32, tag="rep")
        u_sb = big.tile([P, CT, HW], MM_DT, tag="u")
        for ib in range(IB):
            isl = slice(ib * NB, (ib + 1) * NB)
            for ct in range(CT):
                psu = ps_pool.tile([P, NB], F32, tag="mmps")
                for jt in range(JT):
                    nc.tensor.matmul(
                        psu[:],
                        vT_sb[:, jt, ct * P : (ct + 1) * P],
                        est[:, jt, isl],
                        start=(jt == 0), stop=(jt == JT - 1),
                    )
                if ct == 0:
                    ps_r = ps_small.tile([P, NB], F32, tag="pssmall")
                    nc.tensor.matmul(
                        ps_r[:], ones_row[:], recip[:, isl],
                        start=True, stop=True,
                    )
                    nc.vector.tensor_copy(out=rep[:, isl], in_=ps_r[:])
                nc.vector.tensor_tensor(
                    out=u_sb[:, ct, isl], in0=psu[:], in1=rep[:, isl],
                    op=mybir.AluOpType.mult,
                )

        # ---- proj + deferred softmax normalization + residual ----
        for ib in range(IB):
            for ot in range(CT):
                isl = slice(ib * NB, (ib + 1) * NB)
                psp = ps_pool.tile([P, NB], F32, tag="mmps")
                for ct in range(CT):
                    nc.tensor.matmul(
                        psp[:],
                        wp_sb[:, ct, ot * P : (ot + 1) * P],
                        u_sb[:, ct, isl],
                        start=(ct == 0), stop=(ct == CT - 1),
                    )
                ot_t = outp.tile([P, NB], F32, tag="out", name="ot_t")
                nc.vector.tensor_tensor(
                    out=ot_t[:], in0=psp[:], in1=xsb[:, ot, isl],
                    op=mybir.AluOpType.add,
                )
                [nc.sync, nc.scalar, nc.gpsimd, nc.sync][ot].dma_start(
                    out=out.rearrange("(t p) i -> p t i", p=P)[:, ot, isl],
                    in_=ot_t[:],
                )
    return nc


_NC = None


def _get_nc():
    global _NC
    if _NC is None:
        _NC = _build()
    return _NC


def _prep_inputs(x, gn_scale, gn_bias, wq, bq, wk, bk, wv, bv, wproj, bproj):
    f = np.float32
    x = np.ascontiguousarray(x, dtype=f).reshape(B, C, HW)

    def t2(w):  # [o, c] -> [c, o]
        return np.ascontiguousarray(np.asarray(w, dtype=f).T)

    def pt(v):  # [512] -> [128, 4] with v[t*128 + p] at [p, t]
        return np.ascontiguousarray(np.asarray(v, dtype=f).reshape(CT, P).T)

    pidx = np.arange(P)[:, None]
    tidx = np.arange(CT)[None, :]
    grp = 2 * tidx + pidx // GS  # [128, 4] group id per (p, t)
    gmat = np.zeros((P, CT, NG), f)
    hmat = np.zeros((P, CT, P), f)
    for t in range(CT):
        gmat[pidx[:, 0], t, grp[:, t]] = 1.0
        hmat[grp[:, t], t, pidx[:, 0]] = 1.0

    smallcat = np.concatenate(
        [
            pt(np.asarray(bq, dtype=f) * np.float32(SCALE)),
            pt(bk), pt(bproj), pt(gn_scale), pt(gn_bias),
            gmat.reshape(P, CT * NG), hmat.reshape(P, CT * P),
        ],
        axis=1,
    )
    shared = {
        "wqT": t2(wq), "wkT": t2(wk), "wvT": t2(wv), "wpT": t2(wproj),
        "bv": np.ascontiguousarray(np.asarray(bv, dtype=f)),
        "smallcat": np.ascontiguousarray(smallcat),
    }
    return [dict(shared, x=np.ascontiguousarray(x[b])) for b in range(B)]


def _run(inputs, **kw):
    nc = _get_nc()
    in_maps = _prep_inputs(**inputs)
    return run_bass_kernel_spmd(nc, in_maps, core_ids=list(range(B)), **kw)


def kernel(**inputs) -> np.ndarray:
    res = _run(inputs)
    out = np.stack([res.results[b]["out"] for b in range(B)])
    return out.reshape(B, C, HH, WW).astype(np.float32)
